# revision 23
# baseline (speedup 1.0000x reference)
"""Trainium2 Bass kernel for nn_AugmentedODE (B=64, N=P=512), 8-core data parallel.

Per batch the reference computes (7 matmuls of 512^3):
    Omega   = 0.5*(A - A^T)
    du      = u @ Omega + G - u @ (u^T G)
    S       = lam @ G^T
    dlam    = lam @ A + (S + S^T) @ u

Restructured to 5 matmuls + 1 PE transpose set per batch:
    UTG = u^T G                      (bf16:  lhsT=u,    rhs=G)
    W   = 0.5*(A - A^T) - UTG        (DVE; A - A^T precomputed host-side, fp8)
    du  = u @ W + G                  (bf16:  lhsT=u^T,  rhs=W; +G fused in DVE)
    S   = lam @ G^T                  (bf16:  lhsT=lam^T, rhs=G^T)
    C   = S + S^T                    (bf16 PE transpose + DVE add)
    dlam= lam @ A + C @ u            (lam@A in fp8 DoubleRow at 2x rate; C@u bf16;
                                      both accumulated into one PSUM group)

Rel-err budget is 2e-2 (Frobenius); measured ~3.7e-3 for this mix.  The four
magnitude-dominant matmuls (UTG, u@W, S, C@u) stay bf16 and run at the PE's
bf16 roofline (1 row/cycle, ~216 ns per 128x128x512 matmul); fp8 on any of
them measures ~3e-2 error, so only lam@A (~3% of |dlam|) and the skew term
(~5% of |W|) are fp8.

All operands are pre-packed on the host into the exact SBUF layout
([128 partitions, kblock, 512] with k-blocks contiguous per partition) and
concatenated into three blobs per batch, so every DMA line is multi-KB
contiguous on both sides.  Dependency tracking is per-tile, so batch 0 uses
peeled per-piece tiles whose DMAs are sequenced on one queue in consumption
order; batches 1-3 stream whole blobs on the same queue behind them; later
batches prefetch on parallel queues, gated naturally by the 4-deep input
rings.  Batches are processed in PAIRS with sections interleaved
(M1(b), M1(b+1), M5(b), M5(b+1), ...) so every section boundary is followed
by independent work and cross-engine handoff latencies stay off the PE
critical path.  Outputs are written bf16 and upcast on the host.
"""
import numpy as np
import ml_dtypes

import concourse.bass as bass
import concourse.mybir as mybir
import concourse.tile as tile
from concourse import bacc
from concourse.bass_utils import run_bass_kernel_spmd
from concourse.masks import make_identity

F32 = mybir.dt.float32
F32R = mybir.dt.float32r
BF16 = mybir.dt.bfloat16
F8 = mybir.dt.float8e4
AOP = mybir.AluOpType
DR = mybir.MatmulPerfMode.DoubleRow

NP_BF16 = ml_dtypes.bfloat16
NP_F8 = ml_dtypes.float8_e4m3

B, N, P = 64, 512, 512
NCORES = 8
BLOC = B // NCORES          # batches per core
KB = 4                      # 512 = 4 k-blocks of 128
CH = 4                      # 4 output chunks of 128 rows
WARMUP_MM = 6


def _build_nc():
    nc = bacc.Bacc("TRN2", target_bir_lowering=False, debug=False,
                   num_devices=NCORES)

    # in1: interleaved [u0,g0,u1,g1,u2,g2,u3,g3]
    # in2: interleaved [lamt0,gt0,...,lamt3,gt3, ut0..ut3]
    # in8: amat=A-A^T(0:4) | a(4:8) | lamt8(8:12), fp8e4
    d_in1 = nc.declare_dram_parameter("in1", [BLOC, 128, 2 * KB, P], BF16,
                                      isOutput=False)
    d_in2 = nc.declare_dram_parameter("in2", [BLOC, 128, 3 * KB, P], BF16,
                                      isOutput=False)
    d_in8 = nc.declare_dram_parameter("in8", [BLOC, 128, 3 * KB, P], F8,
                                      isOutput=False)
    d_du = nc.declare_dram_parameter("du", [BLOC, 128, KB, P], BF16,
                                     isOutput=True)
    d_dlam = nc.declare_dram_parameter("dlam", [BLOC, 128, KB, P], BF16,
                                       isOutput=True)

    with tile.TileContext(nc) as tc:
        with (
            tc.tile_pool(name="const", bufs=1) as constp,
            tc.tile_pool(name="ins", bufs=4) as insp,
            tc.tile_pool(name="mid", bufs=2) as midp,
            tc.tile_pool(name="outs", bufs=2) as outsp,
            tc.tile_pool(name="psum", bufs=6, space="PSUM") as psum,
        ):
            # identity + warm-up source, gpsimd-only so nothing blocks them
            identf = constp.tile([128, 128], F32)
            make_identity(nc, identf[:])
            identb = constp.tile([128, 128], BF16)
            nc.gpsimd.tensor_copy(identb[:], identf[:])
            wsrc = constp.tile([128, 512], BF16)
            nc.gpsimd.memset(wsrc[:].bitcast(F32), 0.0)

            # batch-0 peeled tiles: per-piece DMAs on ONE queue in consumption
            # order (dep tracking is per-tile, so fine pieces start compute early)
            h1a = insp.tile([128, 2, P], BF16, tag="h1a", bufs=1)      # u0,g0
            h1b = insp.tile([128, 6, P], BF16, tag="h1b", bufs=1)      # u1..g3
            h8a = insp.tile([128, KB, P], F8, tag="h8a", bufs=1)       # amat
            h2a = insp.tile([128, 2 * KB, P], BF16, tag="h2a", bufs=1)  # lamt/gt
            h2c = insp.tile([128, KB, P], BF16, tag="h2c", bufs=1)     # ut
            h8b = insp.tile([128, 2 * KB, P], F8, tag="h8b", bufs=1)   # a|lamt8
            nc.sync.dma_start(h1a[:], d_in1[0][:, 0:2])
            nc.sync.dma_start(h1b[:], d_in1[0][:, 2:2 * KB])
            nc.sync.dma_start(h8a[:], d_in8[0][:, 0:KB])
            nc.sync.dma_start(h2a[:], d_in2[0][:, 0:2 * KB])
            nc.sync.dma_start(h2c[:], d_in2[0][:, 2 * KB:3 * KB])
            nc.sync.dma_start(h8b[:], d_in8[0][:, KB:3 * KB])

            # HAM warm-up: dummy matmuls during the head DMA wait so the
            # first real batch runs at full clock instead of the cold p-state
            warm_ps = psum.tile([128, 512], F32, tag="ps")
            for i in range(WARMUP_MM):
                nc.tensor.matmul(warm_ps[:], identb[:], wsrc[:],
                                 start=True, stop=True)

            def mk_views(b):
                """Allocate input tiles (b>0), emit their DMAs, return accessors."""
                if b == 0:
                    return dict(
                        Un=lambda k: h1a[:, 0] if k == 0 else h1b[:, 2 * (k - 1)],
                        Gn=lambda k: h1a[:, 1] if k == 0 else h1b[:, 2 * k - 1],
                        LT=lambda k: h2a[:, 2 * k],
                        GT=lambda k: h2a[:, 2 * k + 1],
                        UT=lambda k: h2c[:, k],
                        AM8=lambda k: h8a[:, k],
                        A8pair=lambda j: h8b[:, 2 * j:2 * j + 2],
                        L8pair=lambda j, r: h8b[:, KB + 2 * j:KB + 2 * j + 2,
                                                r * 128:(r + 1) * 128],
                    )
                in1 = insp.tile([128, 2 * KB, P], BF16, tag="in1",
                                name=f"in1_{b}")
                in2 = insp.tile([128, 3 * KB, P], BF16, tag="in2",
                                name=f"in2_{b}")
                in8 = insp.tile([128, 3 * KB, P], F8, tag="in8",
                                name=f"in8_{b}")
                if b <= 4:
                    # still in the head: keep everything ordered on sync
                    # (b>=5 is ring-gated behind live tiles, so parallel
                    # queues can't steal head bandwidth)
                    nc.sync.dma_start(in1[:], d_in1[b])
                    nc.sync.dma_start(in8[:], d_in8[b])
                    nc.sync.dma_start(in2[:], d_in2[b])
                else:
                    nc.sync.dma_start(in1[:], d_in1[b])
                    nc.gpsimd.dma_start(in2[:], d_in2[b])
                    nc.gpsimd.dma_start(in8[:], d_in8[b])
                return dict(
                    Un=lambda k, t=in1: t[:, 2 * k],
                    Gn=lambda k, t=in1: t[:, 2 * k + 1],
                    LT=lambda k, t=in2: t[:, 2 * k],
                    GT=lambda k, t=in2: t[:, 2 * k + 1],
                    UT=lambda k, t=in2: t[:, 2 * KB + k],
                    AM8=lambda k, t=in8: t[:, k],
                    A8pair=lambda j, t=in8: t[:, KB + 2 * j:KB + 2 * j + 2],
                    L8pair=lambda j, r, t=in8: t[:, 2 * KB + 2 * j:
                                                 2 * KB + 2 * j + 2,
                                                 r * 128:(r + 1) * 128],
                )

            def sec_m1(b, V):
                """M1: UTG = u^T G (k-outer) ; W = 0.5*amat - UTG (DVE)."""
                w_sb = midp.tile([128, KB, P], BF16, tag="w", name=f"w_{b}")
                utg = [psum.tile([128, P], F32, tag="ps", name=f"utg{b}_{r}")
                       for r in range(CH)]
                for k in range(KB):
                    for r in range(CH):
                        nc.tensor.matmul(utg[r][:],
                                         V["Un"](k)[:, r * 128:(r + 1) * 128],
                                         V["Gn"](k)[:], start=(k == 0),
                                         stop=(k == KB - 1))
                for r in range(CH):
                    nc.vector.scalar_tensor_tensor(w_sb[:, r], V["AM8"](r)[:],
                                                   0.5, utg[r][:], AOP.mult,
                                                   AOP.subtract)
                return w_sb

            def sec_m5(b, V):
                """M5: S = lam @ G^T (k-outer)."""
                s_sb = midp.tile([128, KB, N], BF16, tag="s", name=f"s_{b}")
                s_ps = [psum.tile([128, N], F32, tag="ps", name=f"s{b}_{r}")
                        for r in range(CH)]
                for k in range(KB):
                    for r in range(CH):
                        nc.tensor.matmul(s_ps[r][:],
                                         V["LT"](k)[:, r * 128:(r + 1) * 128],
                                         V["GT"](k)[:], start=(k == 0),
                                         stop=(k == KB - 1))
                for r in range(CH):
                    nc.scalar.copy(s_sb[:, r], s_ps[r][:])
                return s_sb

            def sec_m23(b, V, w_sb):
                """M23: du = u @ W + G, stored out."""
                du_sb = outsp.tile([128, KB, P], BF16, tag="du", name=f"du_{b}")
                for r in range(CH):
                    ps = psum.tile([128, P], F32, tag="ps", name=f"du{b}_{r}")
                    for k in range(KB):
                        nc.tensor.matmul(ps[:],
                                         V["UT"](k)[:, r * 128:(r + 1) * 128],
                                         w_sb[:, k], start=(k == 0),
                                         stop=(k == KB - 1))
                    nc.vector.tensor_tensor(du_sb[:, r], ps[:], V["Gn"](r)[:],
                                            AOP.add)
                    if b == BLOC - 1:
                        nc.scalar.dma_start(d_du[b][:, r], du_sb[:, r])
                if b < BLOC - 1:
                    nc.scalar.dma_start(d_du[b], du_sb[:])

            def sec_tail(b, V, s_sb):
                """C = S + S^T, then dlam = lam @ A (fp8 DR) + C @ u."""
                coup_sb = midp.tile([128, KB, N], BF16, tag="coup",
                                    name=f"coup_{b}")
                for r in range(CH):
                    tps = psum.tile([128, N], BF16, tag="tps", bufs=2,
                                    name=f"tps{b}_{r}")
                    for c in range(KB):
                        nc.tensor.transpose(tps[:, c * 128:(c + 1) * 128],
                                            s_sb[:, c, r * 128:(r + 1) * 128],
                                            identb[:])
                    nc.vector.tensor_tensor(coup_sb[:, r], tps[:], s_sb[:, r],
                                            AOP.add)

                dlam_sb = outsp.tile([128, KB, P], BF16, tag="dlam",
                                     name=f"dlam_{b}")
                dlam_ps = [psum.tile([128, P], F32, tag="ps", name=f"dl{b}_{r}")
                           for r in range(CH)]
                for r in range(CH):
                    for j in range(2):
                        nc.tensor.matmul(dlam_ps[r][:], V["L8pair"](j, r),
                                         V["A8pair"](j), perf_mode=DR,
                                         start=(j == 0), stop=False,
                                         skip_group_check=True)
                for r in range(CH):
                    ps = dlam_ps[r]
                    for k in range(KB):
                        nc.tensor.matmul(ps[:],
                                         coup_sb[:, k, r * 128:(r + 1) * 128],
                                         V["Un"](k)[:], start=False,
                                         stop=(k == KB - 1),
                                         skip_group_check=True)
                    # alternate copy engines so the copies drain in parallel
                    if r % 2 == 0:
                        nc.vector.tensor_copy(dlam_sb[:, r], ps[:])
                    else:
                        nc.scalar.copy(dlam_sb[:, r], ps[:])
                    if b == BLOC - 1:
                        qq = nc.sync if r % 2 == 0 else nc.scalar
                        qq.dma_start(d_dlam[b][:, r], dlam_sb[:, r])
                if b < BLOC - 1:
                    nc.scalar.dma_start(d_dlam[b], dlam_sb[:])

            # paired batches: interleave sections so every boundary is
            # followed by independent work
            for sb in range(0, BLOC, 2):
                ba, bb = sb, sb + 1
                Va = mk_views(ba)
                Vb = mk_views(bb)
                wa = sec_m1(ba, Va)
                wb = sec_m1(bb, Vb)
                sa = sec_m5(ba, Va)
                sb_ = sec_m5(bb, Vb)
                sec_m23(ba, Va, wa)
                sec_m23(bb, Vb, wb)
                sec_tail(ba, Va, sa)
                sec_tail(bb, Vb, sb_)

    nc.compile()
    return nc


_NC = None


def _pack(x, dt):
    """[BLOC,512,512] -> [BLOC,128,4,512] in SBUF layout (partition-major)."""
    return np.ascontiguousarray(
        x.reshape(BLOC, KB, 128, P).transpose(0, 2, 1, 3).astype(dt))


def _unpack(y):
    """[BLOC,128,4,512] bf16 -> [BLOC,512,512] fp32."""
    return y.transpose(0, 2, 1, 3).reshape(BLOC, N, P).astype(np.float32)


def _make_in_maps(u, lam, A, G):
    u = np.asarray(u, dtype=np.float32)
    lam = np.asarray(lam, dtype=np.float32)
    A = np.asarray(A, dtype=np.float32)
    G = np.asarray(G, dtype=np.float32)

    in_maps = []
    for c in range(NCORES):
        sl = slice(c * BLOC, (c + 1) * BLOC)
        uc, lamc, Ac, Gc = u[sl], lam[sl], A[sl], G[sl]
        At = np.swapaxes(Ac, 1, 2)
        lamt = np.swapaxes(lamc, 1, 2)
        in1 = np.empty((BLOC, 128, 2 * KB, P), dtype=NP_BF16)
        in1[:, :, 0::2] = _pack(uc, NP_BF16)
        in1[:, :, 1::2] = _pack(Gc, NP_BF16)
        in2 = np.empty((BLOC, 128, 3 * KB, P), dtype=NP_BF16)
        in2[:, :, 0:2 * KB:2] = _pack(lamt, NP_BF16)
        in2[:, :, 1:2 * KB:2] = _pack(np.swapaxes(Gc, 1, 2), NP_BF16)
        in2[:, :, 2 * KB:] = _pack(np.swapaxes(uc, 1, 2), NP_BF16)
        # lamt8 is quantized from the bf16 lamt (same value chain as on-device)
        in8 = np.concatenate([_pack(Ac - At, NP_F8), _pack(Ac, NP_F8),
                              _pack(lamt, NP_BF16).astype(NP_F8)], axis=2)
        in_maps.append({"in1": in1, "in2": in2, "in8": in8})
    return in_maps


def kernel(u, lam, A, G, t=None, **_ignored):
    global _NC
    if _NC is None:
        _NC = _build_nc()
    nc = _NC

    in_maps = _make_in_maps(u, lam, A, G)
    res = run_bass_kernel_spmd(nc, in_maps, list(range(NCORES)))
    du = np.concatenate([_unpack(res.results[c]["du"]) for c in range(NCORES)],
                        axis=0)
    dlam = np.concatenate([_unpack(res.results[c]["dlam"])
                           for c in range(NCORES)], axis=0)
    return du, dlam
